# revision 14
# baseline (speedup 1.0000x reference)
"""ConvS2S-style attention block on 8 Trainium2 NeuronCores.

Problem (per batch element b of 8, one NeuronCore each):
    conved_emb = conved[b].T @ w_h2e.T + b_h2e          # [T, E]
    cc         = (conved_emb + x[b]) * scale            # [T, E]
    energy     = cc @ encoder_conved[b].T               # [T, S]
    attention  = softmax(energy, axis=-1)               # [T, S]   (output 0)
    attended   = attention @ encoder_combined[b]        # [T, E]
    hid        = attended @ w_e2h.T + b_e2h             # [T, C]
    out2       = conved[b] + hid.T                      # [C, T]   (output 1)

Device layout strategy (per core):
    ccT   [E, T]  = (s*w_h2e) @ conved + (s*x).T        (phase 1, fp32r matmul)
    energy[T, S]  = ccT.T @ encT  (encT = enc_conved.T) (phase 2, fp32r matmul)
    softmax along free dim, f32 attention DMAed out,
    attnT [S, T] (bf16) via PE-transpose of attention   (phase 2)
    attT  [E, T]  = encC.T @ attnT   (bf16 matmul)      (phase 3)
    out2  [C, T]  = w2T.T @ attT + b2 + conved          (phase 4, fp32r matmul)

SBUF pools are stack-allocated (LIFO) — allocation order below is chosen so
that frees occur in exact reverse order of allocations.
"""

import os

os.environ.setdefault("JAX_PLATFORMS", "axon")

import numpy as np


def _ensure_concourse():
    try:
        import concourse  # noqa: F401
    except ImportError:
        import sys

        for p in ("/opt/trn_rl_repo", "/root/.axon_site/_ro/trn_rl_repo"):
            if os.path.isdir(p):
                sys.path.insert(0, p)
                break


_ensure_concourse()

B, C, T, S, E = 8, 2048, 1024, 1024, 1024
NCORES = 8

_CACHE = {}


def build_nc():
    from contextlib import ExitStack

    import concourse.bacc as bacc
    import concourse.bass as bass  # noqa: F401
    import concourse.tile as tile
    from concourse import mybir

    f32 = mybir.dt.float32
    f32r = mybir.dt.float32r
    bf16 = mybir.dt.bfloat16
    AF = mybir.ActivationFunctionType
    ALU = mybir.AluOpType
    AX = mybir.AxisListType.X

    nc = bacc.Bacc()

    d_conved = nc.declare_dram_parameter("conved", [C, T], f32r, isOutput=False)
    d_xsb = nc.declare_dram_parameter("xsb", [E, T], f32, isOutput=False)
    d_encT = nc.declare_dram_parameter("encT", [E, S], f32r, isOutput=False)
    d_encC = nc.declare_dram_parameter("encC", [S, E], bf16, isOutput=False)
    d_w1T = nc.declare_dram_parameter("w1T", [C, E], f32r, isOutput=False)
    d_w2r = nc.declare_dram_parameter("w2r", [E, C], bf16, isOutput=False)
    d_b2 = nc.declare_dram_parameter("b2", [128, C // 128], f32, isOutput=False)
    d_ident = nc.declare_dram_parameter("ident", [128, 128], f32, isOutput=False)
    d_attn = nc.declare_dram_parameter("attn", [T, S], f32, isOutput=True)
    d_out2 = nc.declare_dram_parameter("out2", [C, T], f32, isOutput=True)

    NC16 = C // 128  # 16
    NE8 = E // 128  # 8
    NS8 = S // 128  # 8
    NT8 = T // 128  # 8

    keep = []  # free-fns we intentionally never call (GC of a tc.tile free
    # closure releases its pool out of order — keep them alive)

    def r(ap):
        return ap.bitcast(f32r)

    with tile.TileContext(nc) as tc:
        # ---------- persistent allocations (bottom of the SBUF stack) ----------
        ident, fr = tc.tile([128, 128], f32, name="ident")
        keep.append(fr)
        b2sb, fr = tc.tile([128, NC16], f32, name="b2sb")
        keep.append(fr)
        stats_ctx = ExitStack()
        stats = stats_ctx.enter_context(tc.tile_pool(name="stats", bufs=24))
        keep.append(stats_ctx)

        psum_ctx = ExitStack()
        psA = psum_ctx.enter_context(tc.tile_pool(name="psA", bufs=6, space="PSUM"))
        psT = psum_ctx.enter_context(tc.tile_pool(name="psT", bufs=2, space="PSUM"))
        keep.append(psum_ctx)

        conved_sb = []
        for i in range(NC16):
            t_, fr = tc.tile([128, T], f32r, name=f"cv{i}")
            keep.append(fr)
            conved_sb.append(t_)

        # ---------- lifetime-ordered allocations (LIFO) ----------
        # closes at ph3 end (outermost of the freed groups)
        attnT_sb = []
        for i in range(NS8):
            t_, fr = tc.tile([128, T], bf16, name=f"attnT{i}")
            keep.append(fr)
            attnT_sb.append(t_)
        encC_sb = []
        for i in range(NS8):
            t_, fr = tc.tile([128, E], bf16, name=f"encC{i}")
            keep.append(fr)
            encC_sb.append(t_)
        # closes at ph2 end
        cc_sb, cc_free = [], []
        for i in range(NE8):
            t_, fr = tc.tile([128, T], f32r, name=f"cc{i}")
            cc_sb.append(t_)
            cc_free.append(fr)
        # closes at ph1 end
        w1_sb, w1_free = [], []
        for i in range(NC16):
            t_, fr = tc.tile([128, E], f32r, name=f"w1_{i}")
            w1_sb.append(t_)
            w1_free.append(fr)

        # input DMAs for phase 1: t2=0 needs conved first halves + all of w1;
        # conved second halves can land during the t2=0 compute pass
        for i in range(NC16):
            nc.sync.dma_start(out=conved_sb[i][:, 0:512], in_=d_conved[i * 128:(i + 1) * 128, 0:512])
            nc.sync.dma_start(out=w1_sb[i][:, :], in_=d_w1T[i * 128:(i + 1) * 128, :])
        for i in range(NC16):
            nc.sync.dma_start(out=conved_sb[i][:, 512:1024], in_=d_conved[i * 128:(i + 1) * 128, 512:1024])

        # ================= Phase 1: ccT[E,T] = w1T.T @ conved + xsb =================
        xstr_ctx = ExitStack()
        xstr = xstr_ctx.enter_context(tc.tile_pool(name="xstr", bufs=4))
        for t2 in range(2):
            tsl = slice(t2 * 512, (t2 + 1) * 512)
            pss = {}
            for e8 in range(NE8):
                pool = psA if e8 < 6 else psT
                tag = "psA" if e8 < 6 else "psT"
                pss[e8] = pool.tile([128, 512], f32, name=f"ps1_{t2}_{e8}", tag=tag)
            for c16 in range(NC16):
                for e8 in range(NE8):
                    nc.tensor.matmul(
                        pss[e8][:, :],
                        w1_sb[c16][:, e8 * 128:(e8 + 1) * 128],
                        conved_sb[c16][:, tsl],
                        start=(c16 == 0),
                        stop=(c16 == NC16 - 1),
                    )
            for e8 in range(NE8):
                xt = xstr.tile([128, 512], f32, name=f"xt{t2}_{e8}", tag="xt")
                nc.sync.dma_start(out=xt[:, :], in_=d_xsb[e8 * 128:(e8 + 1) * 128, tsl])
                nc.vector.tensor_add(cc_sb[e8][:, tsl], pss[e8][:, :], xt[:, :])
        xstr_ctx.close()
        for fr in reversed(w1_free):
            fr()

        # ---------- phase 2 allocations ----------
        encT_sb, encT_free = [], []
        for i in range(NE8):
            t_, fr = tc.tile([128, S], f32r, name=f"encT{i}")
            nc.sync.dma_start(out=t_[:, :], in_=d_encT[i * 128:(i + 1) * 128, :])
            encT_sb.append(t_)
            encT_free.append(fr)
        nc.sync.dma_start(out=ident[:, :], in_=d_ident[:, :])
        stage_ctx = ExitStack()
        stage = stage_ctx.enter_context(tc.tile_pool(name="stage", bufs=3))
        # prefetch encC (needed from mid-ph2 by phase 3)
        for i in range(NS8):
            nc.sync.dma_start(out=encC_sb[i][:, :], in_=d_encC[i * 128:(i + 1) * 128, :])

        # ===== Phase 2: energy -> softmax -> attention out + attnT transpose =====
        for t8 in range(NT8):
            trows = slice(t8 * 128, (t8 + 1) * 128)
            eng = []
            for s2 in range(2):
                pe = psA.tile([128, 512], f32, name=f"eng{t8}_{s2}", tag="psA")
                ssl = slice(s2 * 512, (s2 + 1) * 512)
                for e8 in range(NE8):
                    nc.tensor.matmul(
                        pe[:, :],
                        cc_sb[e8][:, trows],
                        encT_sb[e8][:, ssl],
                        start=(e8 == 0),
                        stop=(e8 == NE8 - 1),
                    )
                eng.append(pe)
            m0 = stats.tile([128, 1], f32, name=f"m0_{t8}", tag="st")
            m1 = stats.tile([128, 1], f32, name=f"m1_{t8}", tag="st")
            nc.vector.reduce_max(m0[:, :], eng[0][:, :], axis=AX)
            nc.vector.reduce_max(m1[:, :], eng[1][:, :], axis=AX)
            nm = stats.tile([128, 1], f32, name=f"nm_{t8}", tag="st")
            nc.vector.tensor_max(m0[:, :], m0[:, :], m1[:, :])
            nc.vector.tensor_scalar_mul(nm[:, :], m0[:, :], -1.0)
            an = stage.tile([128, 1024], f32, name=f"an{t8}", tag="an")
            d0 = stats.tile([128, 1], f32, name=f"d0_{t8}", tag="st")
            d1 = stats.tile([128, 1], f32, name=f"d1_{t8}", tag="st")
            nc.scalar.activation(an[:, 0:512], eng[0][:, :], AF.Exp, bias=nm[:, :], accum_out=d0[:, :])
            nc.scalar.activation(an[:, 512:1024], eng[1][:, :], AF.Exp, bias=nm[:, :], accum_out=d1[:, :])
            nc.vector.tensor_add(d0[:, :], d0[:, :], d1[:, :])
            inv = stats.tile([128, 1], f32, name=f"inv_{t8}", tag="st")
            nc.vector.reciprocal(inv[:, :], d0[:, :])
            nc.vector.tensor_scalar_mul(an[:, :], an[:, :], inv[:, :])
            nc.sync.dma_start(out=d_attn[trows, :], in_=an[:, :])
            for s8 in range(NS8):
                pt = psT.tile([128, 512], f32, name=f"pt{t8}_{s8}", tag="psT")
                nc.tensor.transpose(pt[:, 0:128], an[:, s8 * 128:(s8 + 1) * 128], ident[:, :])
                nc.vector.tensor_copy(attnT_sb[s8][:, trows], pt[:, 0:128])
        stage_ctx.close()
        for fr in reversed(encT_free):
            fr()
        for fr in reversed(cc_free):
            fr()

        # ---------- phase 3/4 allocations ----------
        attT_sb = []
        for i in range(NE8):
            t_, fr = tc.tile([128, T], bf16, name=f"attT{i}")
            keep.append(fr)
            attT_sb.append(t_)
        w2_sb = []
        for i in range(NE8):
            t_, fr = tc.tile([128, C], bf16, name=f"w2r{i}")
            keep.append(fr)
            nc.sync.dma_start(out=t_[:, :], in_=d_w2r[i * 128:(i + 1) * 128, :])
            w2_sb.append(t_)
        nc.sync.dma_start(out=b2sb[:, :], in_=d_b2[:, :])
        osta_ctx = ExitStack()
        osta = osta_ctx.enter_context(tc.tile_pool(name="osta", bufs=4))

        # ===== Phase 3: attT[E,T] = encC.T @ attnT  (bf16 x bf16 -> f32) =====
        for t2 in range(2):
            tsl = slice(t2 * 512, (t2 + 1) * 512)
            for e8 in range(NE8):
                ps = psA.tile([128, 512], f32, name=f"ps3_{t2}_{e8}", tag="psA")
                for s8 in range(NS8):
                    nc.tensor.matmul(
                        ps[:, :],
                        encC_sb[s8][:, e8 * 128:(e8 + 1) * 128],
                        attnT_sb[s8][:, tsl],
                        start=(s8 == 0),
                        stop=(s8 == NS8 - 1),
                    )
                nc.vector.tensor_copy(attT_sb[e8][:, tsl], ps[:, :])

        # ===== Phase 4: out2 = w2T.T @ attT + b2 + conved =====
        for c16 in range(NC16):
            for t2 in range(2):
                tsl = slice(t2 * 512, (t2 + 1) * 512)
                ps = psA.tile([128, 512], f32, name=f"ps4_{c16}_{t2}", tag="psA")
                for e8 in range(NE8):
                    nc.tensor.matmul(
                        ps[:, :],
                        w2_sb[e8][:, c16 * 128:(c16 + 1) * 128],
                        attT_sb[e8][:, tsl],
                        start=(e8 == 0),
                        stop=(e8 == NE8 - 1),
                    )
                ob = osta.tile([128, 512], f32, name=f"ob{c16}_{t2}", tag="ob")
                nc.vector.scalar_tensor_tensor(
                    ob[:, :],
                    ps[:, :],
                    b2sb[:, c16:c16 + 1],
                    conved_sb[c16][:, tsl].bitcast(f32),
                    op0=ALU.add,
                    op1=ALU.add,
                )
                nc.sync.dma_start(out=d_out2[c16 * 128:(c16 + 1) * 128, tsl], in_=ob[:, :])
        # never-released pools must still be sealed so the pool trace resolves
        keep.append(osta_ctx)
        osta.seal()
        stats.seal()
        psA.seal()
        psT.seal()

    _CACHE["keep"] = keep
    if not nc.is_finalized():
        nc.finalize()
    return nc


def _get_nc():
    if "nc" not in _CACHE:
        _CACHE["nc"] = build_nc()
    return _CACHE["nc"]


def _round_f32r(a):
    """Round fp32 -> fp32r (11-bit mantissa, low 12 bits zero), RNE."""
    u = np.ascontiguousarray(a, dtype=np.float32).view(np.uint32)
    r = (u + 0x7FF + ((u >> 12) & 1)) & np.uint32(0xFFFFF000)
    return r.view(np.float32)


def make_in_maps(conved, encoder_conved, encoder_combined, x, scale, w_h2e, b_h2e, w_e2h, b_e2h):
    import ml_dtypes

    f = np.float32
    conved = np.asarray(conved, dtype=f)
    encoder_conved = np.asarray(encoder_conved, dtype=f)
    encoder_combined = np.asarray(encoder_combined, dtype=f)
    x = np.asarray(x, dtype=f)
    s = float(np.asarray(scale, dtype=f).reshape(-1)[0])
    w_h2e = np.asarray(w_h2e, dtype=f)
    b_h2e = np.asarray(b_h2e, dtype=f)
    w_e2h = np.asarray(w_e2h, dtype=f)
    b_e2h = np.asarray(b_e2h, dtype=f)

    w1T = _round_f32r(np.ascontiguousarray(w_h2e.T) * s)  # [C, E]
    w2r = np.ascontiguousarray(w_e2h.T).astype(ml_dtypes.bfloat16)  # [E, C]
    b2m = np.ascontiguousarray(b_e2h.reshape(C // 128, 128).T)  # [128, 16]
    ident = np.eye(128, dtype=f)
    bias1 = (b_h2e * s)[:, None]  # [E, 1]

    in_maps = []
    for b in range(B):
        in_maps.append(
            dict(
                conved=_round_f32r(conved[b]),
                xsb=np.ascontiguousarray(x[b].T) * s + bias1,
                encT=_round_f32r(encoder_conved[b].T),
                encC=np.ascontiguousarray(encoder_combined[b]).astype(ml_dtypes.bfloat16),
                w1T=w1T,
                w2r=w2r,
                b2=b2m,
                ident=ident,
            )
        )
    return in_maps


def run(in_maps, trace=False, **kwargs):
    from concourse.bass_utils import run_bass_kernel_spmd

    nc = _get_nc()
    res = run_bass_kernel_spmd(nc, in_maps, list(range(NCORES)), trace=trace, **kwargs)
    return res


def kernel(conved, encoder_conved, encoder_combined, x, scale, w_h2e, b_h2e, w_e2h, b_e2h):
    in_maps = make_in_maps(
        conved, encoder_conved, encoder_combined, x, scale, w_h2e, b_h2e, w_e2h, b_e2h
    )
    res = run(in_maps)
    attention = np.stack([np.asarray(r["attn"]) for r in res.results])
    attended = np.stack([np.asarray(r["out2"]) for r in res.results])
    return attention, attended


# revision 15
# speedup vs baseline: 1.0664x; 1.0664x over previous
"""ConvS2S-style attention block on 8 Trainium2 NeuronCores.

Problem (per batch element b of 8, one NeuronCore each):
    conved_emb = conved[b].T @ w_h2e.T + b_h2e          # [T, E]
    cc         = (conved_emb + x[b]) * scale            # [T, E]
    energy     = cc @ encoder_conved[b].T               # [T, S]
    attention  = softmax(energy, axis=-1)               # [T, S]   (output 0)
    attended   = attention @ encoder_combined[b]        # [T, E]
    hid        = attended @ w_e2h.T + b_e2h             # [T, C]
    out2       = conved[b] + hid.T                      # [C, T]   (output 1)

Device layout strategy (per core):
    ccT   [E, T]  = (s*w_h2e) @ conved + (s*x).T        (phase 1, fp32r matmul)
    energy[T, S]  = ccT.T @ encT  (encT = enc_conved.T) (phase 2, fp32r matmul)
    softmax along free dim, f32 attention DMAed out,
    attnT [S, T] (bf16) via PE-transpose of attention   (phase 2)
    attT  [E, T]  = encC.T @ attnT   (bf16 matmul)      (phase 3)
    out2  [C, T]  = w2T.T @ attT + b2 + conved          (phase 4, fp32r matmul)

SBUF pools are stack-allocated (LIFO) — allocation order below is chosen so
that frees occur in exact reverse order of allocations.
"""

import os

os.environ.setdefault("JAX_PLATFORMS", "axon")

import numpy as np


def _ensure_concourse():
    try:
        import concourse  # noqa: F401
    except ImportError:
        import sys

        for p in ("/opt/trn_rl_repo", "/root/.axon_site/_ro/trn_rl_repo"):
            if os.path.isdir(p):
                sys.path.insert(0, p)
                break


_ensure_concourse()

B, C, T, S, E = 8, 2048, 1024, 1024, 1024
NCORES = 8

_CACHE = {}


def build_nc():
    from contextlib import ExitStack

    import concourse.bacc as bacc
    import concourse.bass as bass  # noqa: F401
    import concourse.tile as tile
    from concourse import mybir

    f32 = mybir.dt.float32
    f32r = mybir.dt.float32r
    bf16 = mybir.dt.bfloat16
    AF = mybir.ActivationFunctionType
    ALU = mybir.AluOpType
    AX = mybir.AxisListType.X

    nc = bacc.Bacc()

    d_conved = nc.declare_dram_parameter("conved", [C, T], f32r, isOutput=False)
    d_xsb = nc.declare_dram_parameter("xsb", [E, T], f32, isOutput=False)
    d_encT = nc.declare_dram_parameter("encT", [E, S], f32r, isOutput=False)
    d_encC = nc.declare_dram_parameter("encC", [S, E], bf16, isOutput=False)
    d_w1T = nc.declare_dram_parameter("w1T", [C, E], f32r, isOutput=False)
    d_w2r = nc.declare_dram_parameter("w2r", [E, C], bf16, isOutput=False)
    d_b2 = nc.declare_dram_parameter("b2", [128, C // 128], f32, isOutput=False)
    d_ident = nc.declare_dram_parameter("ident", [128, 128], f32, isOutput=False)
    d_attn = nc.declare_dram_parameter("attn", [T, S], f32, isOutput=True)
    d_out2 = nc.declare_dram_parameter("out2", [C, T], f32, isOutput=True)

    NC16 = C // 128  # 16
    NE8 = E // 128  # 8
    NS8 = S // 128  # 8
    NT8 = T // 128  # 8

    keep = []  # free-fns we intentionally never call (GC of a tc.tile free
    # closure releases its pool out of order — keep them alive)

    def r(ap):
        return ap.bitcast(f32r)

    with tile.TileContext(nc) as tc:
        # ---------- persistent allocations (bottom of the SBUF stack) ----------
        ident, fr = tc.tile([128, 128], f32, name="ident")
        keep.append(fr)
        b2sb, fr = tc.tile([128, NC16], f32, name="b2sb")
        keep.append(fr)
        stats_ctx = ExitStack()
        stats = stats_ctx.enter_context(tc.tile_pool(name="stats", bufs=24))
        keep.append(stats_ctx)

        psum_ctx = ExitStack()
        psA = psum_ctx.enter_context(tc.tile_pool(name="psA", bufs=6, space="PSUM"))
        psT = psum_ctx.enter_context(tc.tile_pool(name="psT", bufs=2, space="PSUM"))
        keep.append(psum_ctx)

        conved_sb = []
        for i in range(NC16):
            t_, fr = tc.tile([128, T], f32r, name=f"cv{i}")
            keep.append(fr)
            conved_sb.append(t_)

        # ---------- lifetime-ordered allocations (LIFO) ----------
        # cc is persistent (kept to the end so later pools can stack LIFO)
        cc_sb = []
        for i in range(NE8):
            t_, fr = tc.tile([128, T], f32r, name=f"cc{i}")
            keep.append(fr)
            cc_sb.append(t_)
        # closes at ph1 end
        w1_sb, w1_free = [], []
        for i in range(NC16):
            t_, fr = tc.tile([128, E], f32r, name=f"w1_{i}")
            w1_sb.append(t_)
            w1_free.append(fr)

        # DMA emission mirrors phase-1 consumption order: (conved-half0, w1)
        # pairs, then t2=0 xsb, then conved second halves, then t2=1 xsb.
        xstr_ctx = ExitStack()
        xstr = xstr_ctx.enter_context(tc.tile_pool(name="xstr", bufs=8))
        for i in range(NC16):
            nc.sync.dma_start(out=conved_sb[i][:, 0:512], in_=d_conved[i * 128:(i + 1) * 128, 0:512])
            nc.sync.dma_start(out=w1_sb[i][:, :], in_=d_w1T[i * 128:(i + 1) * 128, :])
        xts = {}
        for t2 in range(2):
            tsl = slice(t2 * 512, (t2 + 1) * 512)
            for e8 in range(NE8):
                xt = xstr.tile([128, 512], f32, name=f"xt{t2}_{e8}", tag="xt")
                nc.sync.dma_start(out=xt[:, :], in_=d_xsb[e8 * 128:(e8 + 1) * 128, tsl])
                xts[(t2, e8)] = xt
            if t2 == 0:
                for i in range(NC16):
                    nc.sync.dma_start(out=conved_sb[i][:, 512:1024], in_=d_conved[i * 128:(i + 1) * 128, 512:1024])

        # ================= Phase 1: ccT[E,T] = w1T.T @ conved + xsb =================
        for t2 in range(2):
            tsl = slice(t2 * 512, (t2 + 1) * 512)
            pss = {}
            for e8 in range(NE8):
                pool = psA if e8 < 6 else psT
                tag = "psA" if e8 < 6 else "psT"
                pss[e8] = pool.tile([128, 512], f32, name=f"ps1_{t2}_{e8}", tag=tag)
            for c16 in range(NC16):
                for e8 in range(NE8):
                    nc.tensor.matmul(
                        pss[e8][:, :],
                        w1_sb[c16][:, e8 * 128:(e8 + 1) * 128],
                        conved_sb[c16][:, tsl],
                        start=(c16 == 0),
                        stop=(c16 == NC16 - 1),
                    )
            for e8 in range(NE8):
                nc.vector.tensor_add(cc_sb[e8][:, tsl], pss[e8][:, :], xts[(t2, e8)][:, :])
        xstr_ctx.close()
        for fr in reversed(w1_free):
            fr()

        # ---------- phase 2 allocations ----------
        attnT_sb = []
        for i in range(NS8):
            t_, fr = tc.tile([128, T], bf16, name=f"attnT{i}")
            keep.append(fr)
            attnT_sb.append(t_)
        encC_sb = []
        for i in range(NS8):
            t_, fr = tc.tile([128, E], bf16, name=f"encC{i}")
            keep.append(fr)
            encC_sb.append(t_)
        encT_sb, encT_free = [], []
        for i in range(NE8):
            t_, fr = tc.tile([128, S], f32r, name=f"encT{i}")
            nc.sync.dma_start(out=t_[:, :], in_=d_encT[i * 128:(i + 1) * 128, :])
            encT_sb.append(t_)
            encT_free.append(fr)
        nc.sync.dma_start(out=ident[:, :], in_=d_ident[:, :])
        stage_ctx = ExitStack()
        stage = stage_ctx.enter_context(tc.tile_pool(name="stage", bufs=3))
        # prefetch encC (needed from mid-ph2 by phase 3)
        for i in range(NS8):
            nc.sync.dma_start(out=encC_sb[i][:, :], in_=d_encC[i * 128:(i + 1) * 128, :])

        # ===== Phase 2: energy -> softmax -> attention out + attnT transpose =====
        for t8 in range(NT8):
            trows = slice(t8 * 128, (t8 + 1) * 128)
            eng = []
            for s2 in range(2):
                pe = psA.tile([128, 512], f32, name=f"eng{t8}_{s2}", tag="psA")
                ssl = slice(s2 * 512, (s2 + 1) * 512)
                for e8 in range(NE8):
                    nc.tensor.matmul(
                        pe[:, :],
                        cc_sb[e8][:, trows],
                        encT_sb[e8][:, ssl],
                        start=(e8 == 0),
                        stop=(e8 == NE8 - 1),
                    )
                eng.append(pe)
            m0 = stats.tile([128, 1], f32, name=f"m0_{t8}", tag="st")
            m1 = stats.tile([128, 1], f32, name=f"m1_{t8}", tag="st")
            nc.vector.reduce_max(m0[:, :], eng[0][:, :], axis=AX)
            nc.vector.reduce_max(m1[:, :], eng[1][:, :], axis=AX)
            nm = stats.tile([128, 1], f32, name=f"nm_{t8}", tag="st")
            nc.vector.tensor_max(m0[:, :], m0[:, :], m1[:, :])
            nc.vector.tensor_scalar_mul(nm[:, :], m0[:, :], -1.0)
            an = stage.tile([128, 1024], f32, name=f"an{t8}", tag="an")
            d0 = stats.tile([128, 1], f32, name=f"d0_{t8}", tag="st")
            d1 = stats.tile([128, 1], f32, name=f"d1_{t8}", tag="st")
            nc.scalar.activation(an[:, 0:512], eng[0][:, :], AF.Exp, bias=nm[:, :], accum_out=d0[:, :])
            nc.scalar.activation(an[:, 512:1024], eng[1][:, :], AF.Exp, bias=nm[:, :], accum_out=d1[:, :])
            nc.vector.tensor_add(d0[:, :], d0[:, :], d1[:, :])
            inv = stats.tile([128, 1], f32, name=f"inv_{t8}", tag="st")
            nc.vector.reciprocal(inv[:, :], d0[:, :])
            nc.vector.tensor_scalar_mul(an[:, :], an[:, :], inv[:, :])
            nc.sync.dma_start(out=d_attn[trows, :], in_=an[:, :])
            for s8 in range(NS8):
                pt = psT.tile([128, 512], f32, name=f"pt{t8}_{s8}", tag="psT")
                nc.tensor.transpose(pt[:, 0:128], an[:, s8 * 128:(s8 + 1) * 128], ident[:, :])
                nc.vector.tensor_copy(attnT_sb[s8][:, trows], pt[:, 0:128])
        stage_ctx.close()
        for fr in reversed(encT_free):
            fr()

        # ---------- phase 3/4 allocations ----------
        attT_sb = []
        for i in range(NE8):
            t_, fr = tc.tile([128, T], bf16, name=f"attT{i}")
            keep.append(fr)
            attT_sb.append(t_)
        w2_sb = []
        for i in range(NE8):
            t_, fr = tc.tile([128, C], bf16, name=f"w2r{i}")
            keep.append(fr)
            nc.sync.dma_start(out=t_[:, :], in_=d_w2r[i * 128:(i + 1) * 128, :])
            w2_sb.append(t_)
        nc.sync.dma_start(out=b2sb[:, :], in_=d_b2[:, :])
        osta_ctx = ExitStack()
        osta = osta_ctx.enter_context(tc.tile_pool(name="osta", bufs=4))

        # ===== Phase 3: attT[E,T] = encC.T @ attnT  (bf16 x bf16 -> f32) =====
        for t2 in range(2):
            tsl = slice(t2 * 512, (t2 + 1) * 512)
            for e8 in range(NE8):
                ps = psA.tile([128, 512], f32, name=f"ps3_{t2}_{e8}", tag="psA")
                for s8 in range(NS8):
                    nc.tensor.matmul(
                        ps[:, :],
                        encC_sb[s8][:, e8 * 128:(e8 + 1) * 128],
                        attnT_sb[s8][:, tsl],
                        start=(s8 == 0),
                        stop=(s8 == NS8 - 1),
                    )
                nc.vector.tensor_copy(attT_sb[e8][:, tsl], ps[:, :])

        # ===== Phase 4: out2 = w2T.T @ attT + b2 + conved =====
        for c16 in range(NC16):
            for t2 in range(2):
                tsl = slice(t2 * 512, (t2 + 1) * 512)
                ps = psA.tile([128, 512], f32, name=f"ps4_{c16}_{t2}", tag="psA")
                for e8 in range(NE8):
                    nc.tensor.matmul(
                        ps[:, :],
                        w2_sb[e8][:, c16 * 128:(c16 + 1) * 128],
                        attT_sb[e8][:, tsl],
                        start=(e8 == 0),
                        stop=(e8 == NE8 - 1),
                    )
                ob = osta.tile([128, 512], f32, name=f"ob{c16}_{t2}", tag="ob")
                nc.vector.scalar_tensor_tensor(
                    ob[:, :],
                    ps[:, :],
                    b2sb[:, c16:c16 + 1],
                    conved_sb[c16][:, tsl].bitcast(f32),
                    op0=ALU.add,
                    op1=ALU.add,
                )
                nc.sync.dma_start(out=d_out2[c16 * 128:(c16 + 1) * 128, tsl], in_=ob[:, :])
        # never-released pools must still be sealed so the pool trace resolves
        keep.append(osta_ctx)
        osta.seal()
        stats.seal()
        psA.seal()
        psT.seal()

    _CACHE["keep"] = keep
    if not nc.is_finalized():
        nc.finalize()
    return nc


def _get_nc():
    if "nc" not in _CACHE:
        _CACHE["nc"] = build_nc()
    return _CACHE["nc"]


def _round_f32r(a):
    """Round fp32 -> fp32r (11-bit mantissa, low 12 bits zero), RNE."""
    u = np.ascontiguousarray(a, dtype=np.float32).view(np.uint32)
    r = (u + 0x7FF + ((u >> 12) & 1)) & np.uint32(0xFFFFF000)
    return r.view(np.float32)


def make_in_maps(conved, encoder_conved, encoder_combined, x, scale, w_h2e, b_h2e, w_e2h, b_e2h):
    import ml_dtypes

    f = np.float32
    conved = np.asarray(conved, dtype=f)
    encoder_conved = np.asarray(encoder_conved, dtype=f)
    encoder_combined = np.asarray(encoder_combined, dtype=f)
    x = np.asarray(x, dtype=f)
    s = float(np.asarray(scale, dtype=f).reshape(-1)[0])
    w_h2e = np.asarray(w_h2e, dtype=f)
    b_h2e = np.asarray(b_h2e, dtype=f)
    w_e2h = np.asarray(w_e2h, dtype=f)
    b_e2h = np.asarray(b_e2h, dtype=f)

    w1T = _round_f32r(np.ascontiguousarray(w_h2e.T) * s)  # [C, E]
    w2r = np.ascontiguousarray(w_e2h.T).astype(ml_dtypes.bfloat16)  # [E, C]
    b2m = np.ascontiguousarray(b_e2h.reshape(C // 128, 128).T)  # [128, 16]
    ident = np.eye(128, dtype=f)
    bias1 = (b_h2e * s)[:, None]  # [E, 1]

    in_maps = []
    for b in range(B):
        in_maps.append(
            dict(
                conved=_round_f32r(conved[b]),
                xsb=np.ascontiguousarray(x[b].T) * s + bias1,
                encT=_round_f32r(encoder_conved[b].T),
                encC=np.ascontiguousarray(encoder_combined[b]).astype(ml_dtypes.bfloat16),
                w1T=w1T,
                w2r=w2r,
                b2=b2m,
                ident=ident,
            )
        )
    return in_maps


def run(in_maps, trace=False, **kwargs):
    from concourse.bass_utils import run_bass_kernel_spmd

    nc = _get_nc()
    res = run_bass_kernel_spmd(nc, in_maps, list(range(NCORES)), trace=trace, **kwargs)
    return res


def kernel(conved, encoder_conved, encoder_combined, x, scale, w_h2e, b_h2e, w_e2h, b_e2h):
    in_maps = make_in_maps(
        conved, encoder_conved, encoder_combined, x, scale, w_h2e, b_h2e, w_e2h, b_e2h
    )
    res = run(in_maps)
    attention = np.stack([np.asarray(r["attn"]) for r in res.results])
    attended = np.stack([np.asarray(r["out2"]) for r in res.results])
    return attention, attended
